# revision 30
# baseline (speedup 1.0000x reference)
"""VQ codebook (E8P-style, 256 codes, dim 8) nearest-neighbor kernel for TRN2.

Contract: kernel(**inputs) takes FULL inputs (X [1048576, 8] f32, grid
[256, 8] f32, grid_norm [256] f32) and returns (Xq [1048576, 8] f32,
idx [1048576] uint8), matching the reference
    scores = 2 X @ grid.T - grid_norm ; idx = argmax(scores) ; Xq = grid[idx].

Sharding: data-parallel over the leading N dimension across 8 NeuronCores
(hardcoded). Each core runs the same SPMD program on its 131072-row shard;
results are concatenated on the host.

Algorithm — fully-arithmetic algebraic E8P decode (no 256-wide matmul, no
scans, no gathers). The codebook from reference._build_grid() is exactly
  ints  (113): entries {-1,0,1}, 0 or 2 nonzeros, even sum      idx 0..112
  halfs (128): all +-0.5 with an even number of minus signs     idx 113..240
  extras (15): four-ones masks {5 triples} x {last in 5,6,7}    idx 241..255
so per row the argmax reduces to 4 family candidates computed with cheap
row-wise vector ops:
  A0: zero code, score 0, idx 56
  A2: +-1 at the top-2 |x| positions, score 2(t1+t2)-2
  B : 0.5*sign(x) with a parity flip at argmin|x|,
      score sum|x| - 2 - (odd ? 2 min|x| : 0)
  C : score 2(maxT+maxL) - 4 over the 5 triples / 3 last dims
Both outputs are then reconstructed arithmetically:
  idx: closed-form table ranks (cartesian-order rank formulas validated
       against the table) selected by family;
  Xq : the winning code vector rebuilt from signs/masks.

Layout: partition p owns rows [p*1024, (p+1)*1024); blocks of R=128 rows
per partition are processed with [128, R, 8]-shaped fp32 ops so every DVE
instruction covers 16384 rows (instruction overhead amortized).
"""

import numpy as np

N_CORES = 8
N_TOTAL = 1048576
N_PER_CORE = N_TOTAL // N_CORES  # 131072
P = 128
D = 8


def _build_program_v3(n_rows, R=128):
    import concourse.bass as bass
    import concourse.bacc as bacc
    import concourse.mybir as mybir
    import concourse.tile as tile
    from concourse.alu_op_type import AluOpType as op
    from contextlib import ExitStack

    f32 = mybir.dt.float32
    i32 = mybir.dt.int32
    u8 = mybir.dt.uint8
    AF = mybir.ActivationFunctionType
    X_ = mybir.AxisListType.X

    rpp = n_rows // P
    assert n_rows % P == 0 and rpp % R == 0
    n_blocks = rpp // R

    nc = bacc.Bacc(trn_type="TRN2", target_bir_lowering=False, debug=False)

    x = nc.dram_tensor("x", [n_rows, D], f32, kind="ExternalInput").ap()
    # consts [128, 32]: 0:8 = WC (idxC weights), 8:16 = W7 (idxB weights),
    # 16:24 = iota, 24:32 = iota^2
    cst = nc.dram_tensor("cst", [P, 32], f32, kind="ExternalInput").ap()

    xq = nc.dram_tensor("xq", [n_rows, D], f32, kind="ExternalOutput").ap()
    idx_out = nc.dram_tensor("idx_out", [n_rows], u8, kind="ExternalOutput").ap()

    xv = x.rearrange("(p r) d -> p r d", p=P)
    xqv = xq.rearrange("(p r) d -> p r d", p=P)
    idxv = idx_out.rearrange("(p r) -> p r", p=P)

    cst_sb = nc.alloc_sbuf_tensor("cst_sb", [P, 32], f32).ap()
    wcc = cst_sb[:, 0:8]
    w7c = cst_sb[:, 8:16]
    ioc = cst_sb[:, 16:24]
    io2c = cst_sb[:, 24:32]

    def bc8(t):  # [P, R] -> [P, R, 8]
        return t.unsqueeze(2).to_broadcast([P, R, 8])

    def cc8(c):  # [P, 8] const -> [P, R, 8]
        return c.unsqueeze(1).to_broadcast([P, R, 8])

    with tile.TileContext(nc) as tc, ExitStack() as ctx:
        nc.sync.dma_start(cst_sb[:], cst[:])

        xb_pool = ctx.enter_context(tc.tile_pool(name="xb", bufs=2))
        v8_pool = ctx.enter_context(tc.tile_pool(name="v8", bufs=1))
        sc_pool = ctx.enter_context(tc.tile_pool(name="sc", bufs=1))
        out_pool = ctx.enter_context(tc.tile_pool(name="outp", bufs=2))

        V = nc.vector
        A = nc.scalar
        G = nc.gpsimd

        for blk in range(n_blocks):
            xb = xb_pool.tile([P, R, D], f32)
            nc.sync.dma_start(xb[:], xv[:, blk * R : (blk + 1) * R, :])

            def t8(tag):
                return v8_pool.tile([P, R, D], f32, tag=tag, name=f"t8_{tag}")

            def tr(tag):
                return sc_pool.tile([P, R], f32, tag=tag, name=f"tr_{tag}")

            ax = t8("ax")
            A.activation(ax[:], xb[:], AF.Abs)

            sumabs = tr("sumabs")
            V.tensor_reduce(sumabs[:], ax[:], axis=X_, op=op.add)
            t1 = tr("t1")
            V.tensor_reduce(t1[:], ax[:], axis=X_, op=op.max)
            mn = tr("mn")
            V.tensor_reduce(mn[:], ax[:], axis=X_, op=op.min)

            neg = t8("neg")
            V.tensor_single_scalar(neg[:], xb[:], 0.0, op=op.is_lt)
            nneg = tr("nneg")
            V.tensor_reduce(nneg[:], neg[:], axis=X_, op=op.add)
            oddi = sc_pool.tile([P, R], i32, tag="oddi", name="oddi")
            V.tensor_copy(oddi[:], nneg[:])
            V.tensor_single_scalar(oddi[:], oddi[:], 1, op=op.bitwise_and)
            odd = tr("odd")
            V.tensor_copy(odd[:], oddi[:])

            # top-2 of |x|
            ltm = t8("ltm")
            V.tensor_tensor(ltm[:], ax[:], bc8(t1), op=op.is_lt)
            axl = t8("axl")
            G.tensor_tensor(axl[:], ax[:], ltm[:], op=op.mult)
            t2 = tr("t2")
            V.tensor_reduce(t2[:], axl[:], axis=X_, op=op.max)
            mask2 = t8("mask2")
            V.tensor_tensor(mask2[:], ax[:], bc8(t2), op=op.is_ge)

            sA2 = tr("sA2")
            V.tensor_add(sA2[:], t1[:], t2[:])
            V.tensor_scalar(sA2[:], sA2[:], 2.0, -2.0, op0=op.mult, op1=op.add)

            sB = tr("sB")
            V.tensor_mul(sB[:], odd[:], mn[:])
            V.tensor_scalar(sB[:], sB[:], -2.0, -2.0, op0=op.mult, op1=op.add)
            V.tensor_add(sB[:], sB[:], sumabs[:])

            # C family: triple sums in tb[0:5], scratch in tb[5:8]
            tb = t8("tb")
            V.tensor_add(tb[:, :, 5], xb[:, :, 0], xb[:, :, 1])
            V.tensor_add(tb[:, :, 6], xb[:, :, 3], xb[:, :, 4])
            V.tensor_add(tb[:, :, 7], xb[:, :, 1], xb[:, :, 2])
            V.tensor_add(tb[:, :, 0], tb[:, :, 5], xb[:, :, 2])
            V.tensor_add(tb[:, :, 1], tb[:, :, 5], xb[:, :, 4])
            V.tensor_add(tb[:, :, 2], tb[:, :, 6], xb[:, :, 0])
            V.tensor_add(tb[:, :, 3], tb[:, :, 6], xb[:, :, 2])
            V.tensor_add(tb[:, :, 4], tb[:, :, 7], xb[:, :, 3])
            maxT = tr("maxT")
            V.tensor_reduce(maxT[:], tb[:, :, 0:5], axis=X_, op=op.max)
            maxL = tr("maxL")
            V.tensor_reduce(maxL[:], xb[:, :, 5:8], axis=X_, op=op.max)
            mb = t8("mb")
            V.tensor_tensor(
                mb[:, :, 0:5], tb[:, :, 0:5], bc8(maxT)[:, :, 0:5], op=op.is_ge
            )
            V.tensor_tensor(
                mb[:, :, 5:8], xb[:, :, 5:8], bc8(maxL)[:, :, 5:8], op=op.is_ge
            )
            wb = t8("wb")
            V.tensor_tensor(wb[:], mb[:], cc8(wcc), op=op.mult)
            idxC = tr("idxC")
            V.tensor_reduce(idxC[:], wb[:], axis=X_, op=op.add)
            sC = tr("sC")
            V.tensor_add(sC[:], maxT[:], maxL[:])
            V.tensor_scalar(sC[:], sC[:], 2.0, -4.0, op0=op.mult, op1=op.add)

            # winner score
            w = tr("w")
            V.tensor_max(w[:], sA2[:], sB[:])
            V.tensor_max(w[:], w[:], sC[:])
            V.tensor_single_scalar(w[:], w[:], 0.0, op=op.max)

            # sigma & parity flip (affines on ACT, plain mults on Pool)
            sg = t8("sg")
            A.activation(sg[:], neg[:], AF.Copy, bias=1.0, scale=-2.0)
            flip = t8("flip")
            V.tensor_tensor(flip[:], ax[:], bc8(mn), op=op.is_equal)
            fo = t8("fo")
            V.tensor_tensor(fo[:], flip[:], bc8(odd), op=op.mult)
            A.activation(fo[:], fo[:], AF.Copy, bias=1.0, scale=-2.0)
            sgf = t8("sgf")
            G.tensor_tensor(sgf[:], sg[:], fo[:], op=op.mult)
            va = t8("va")
            G.tensor_tensor(va[:], sg[:], mask2[:], op=op.mult)

            # family masks, tie priority A0 > A2 > B > C
            mA = tr("mA")
            V.tensor_tensor(mA[:], sA2[:], w[:], op=op.is_ge)
            mB = tr("mB")
            V.tensor_tensor(mB[:], sB[:], w[:], op=op.is_ge)
            mZ = tr("mZ")
            V.tensor_single_scalar(mZ[:], w[:], 0.0, op=op.is_le)
            nA = tr("nA")
            V.tensor_scalar(nA[:], mA[:], -1.0, 1.0, op0=op.mult, op1=op.add)
            nB = tr("nB")
            V.tensor_scalar(nB[:], mB[:], -1.0, 1.0, op0=op.mult, op1=op.add)
            nZ = tr("nZ")
            V.tensor_scalar(nZ[:], mZ[:], -1.0, 1.0, op0=op.mult, op1=op.add)
            fA = tr("fA")
            V.tensor_mul(fA[:], mA[:], nZ[:])
            fB = tr("fB")
            V.tensor_mul(fB[:], mB[:], nA[:])
            V.tensor_mul(fB[:], fB[:], nZ[:])
            fC = tr("fC")
            V.tensor_mul(fC[:], nB[:], nA[:])
            V.tensor_mul(fC[:], fC[:], nZ[:])

            # idxB = 113 + dot((sgf+1)/2 [0:7], 2^(6-d))
            #      = 0.5*dot(sgf, W7) + 176.5
            kb8 = t8("kb8")
            V.tensor_tensor(kb8[:], sgf[:], cc8(w7c), op=op.mult)
            idxB = tr("idxB")
            V.tensor_reduce(idxB[:], kb8[:], axis=X_, op=op.add)
            V.tensor_scalar(idxB[:], idxB[:], 0.5, 176.5, op0=op.mult, op1=op.add)

            # idxA: positions/signs of the top-2, closed-form rank
            dots = t8("dots")
            V.tensor_tensor(dots[:], mask2[:], cc8(ioc), op=op.mult)
            S = tr("S")
            V.tensor_reduce(S[:], dots[:], axis=X_, op=op.add)
            V.tensor_tensor(dots[:], mask2[:], cc8(io2c), op=op.mult)
            Q = tr("Q")
            V.tensor_reduce(Q[:], dots[:], axis=X_, op=op.add)
            sumva = tr("sumva")
            V.tensor_reduce(sumva[:], va[:], axis=X_, op=op.add)
            V.tensor_tensor(dots[:], va[:], cc8(ioc), op=op.mult)
            dif = tr("dif")
            V.tensor_reduce(dif[:], dots[:], axis=X_, op=op.add)

            disc = tr("disc")
            V.tensor_mul(disc[:], S[:], S[:])
            V.tensor_scalar(disc[:], disc[:], -1.0, 0.0, op0=op.mult, op1=op.add)
            qq = tr("qq")
            V.tensor_scalar(qq[:], Q[:], 2.0, 0.0, op0=op.mult, op1=op.add)
            V.tensor_add(disc[:], disc[:], qq[:])
            V.tensor_single_scalar(disc[:], disc[:], 0.0, op=op.max)
            Dd = tr("Dd")
            A.activation(Dd[:], disc[:], AF.Sqrt)
            aa = tr("aa")
            V.tensor_sub(aa[:], S[:], Dd[:])
            V.tensor_scalar(aa[:], aa[:], 0.5, 0.0, op0=op.mult, op1=op.add)
            bb = tr("bb")
            V.tensor_add(bb[:], S[:], Dd[:])
            V.tensor_scalar(bb[:], bb[:], 0.5, 0.0, op0=op.mult, op1=op.add)

            sv2 = tr("sv2")
            V.tensor_mul(sv2[:], sumva[:], sumva[:])
            msame = tr("msame")
            V.tensor_single_scalar(msame[:], sv2[:], 4.0, op=op.is_ge)
            sopp = tr("sopp")
            V.tensor_single_scalar(sopp[:], dif[:], 0.0, op=op.is_ge)
            V.tensor_scalar(sopp[:], sopp[:], -2.0, 1.0, op0=op.mult, op1=op.add)
            hsv = tr("hsv")
            V.tensor_scalar(hsv[:], sumva[:], 0.5, 0.0, op0=op.mult, op1=op.add)
            V.tensor_sub(hsv[:], hsv[:], sopp[:])
            V.tensor_mul(hsv[:], hsv[:], msame[:])
            sa = tr("sa")
            V.tensor_add(sa[:], sopp[:], hsv[:])
            sb_ = tr("sb_")
            V.tensor_sub(sb_[:], sumva[:], sa[:])
            spa = tr("spa")
            V.tensor_single_scalar(spa[:], sa[:], 0.0, op=op.is_gt)
            spb = tr("spb")
            V.tensor_single_scalar(spb[:], sb_[:], 0.0, op=op.is_gt)

            # rank = a(15-a) + spa*(2(7-a) + 2(7-a)(6-a) + 1)
            #        + (b-a-1) + spb*(1 + 2(7-b))
            f15 = tr("f15")
            V.tensor_scalar(f15[:], aa[:], -1.0, 15.0, op0=op.mult, op1=op.add)
            rank = tr("rank")
            V.tensor_mul(rank[:], aa[:], f15[:])
            s7a = tr("s7a")
            V.tensor_scalar(s7a[:], aa[:], -1.0, 7.0, op0=op.mult, op1=op.add)
            s6a = tr("s6a")
            V.tensor_scalar(s6a[:], aa[:], -1.0, 6.0, op0=op.mult, op1=op.add)
            pa = tr("pa")
            V.tensor_mul(pa[:], s7a[:], s6a[:])
            V.tensor_add(pa[:], pa[:], s7a[:])
            V.tensor_scalar(pa[:], pa[:], 2.0, 1.0, op0=op.mult, op1=op.add)
            V.tensor_mul(pa[:], pa[:], spa[:])
            V.tensor_add(rank[:], rank[:], pa[:])
            p3 = tr("p3")
            V.tensor_sub(p3[:], bb[:], aa[:])
            V.tensor_scalar(p3[:], p3[:], 1.0, -1.0, op0=op.mult, op1=op.add)
            V.tensor_add(rank[:], rank[:], p3[:])
            pb = tr("pb")
            V.tensor_scalar(pb[:], bb[:], -2.0, 15.0, op0=op.mult, op1=op.add)
            V.tensor_mul(pb[:], pb[:], spb[:])
            V.tensor_add(rank[:], rank[:], pb[:])

            # final idx = fA*rank + fB*idxB + fC*idxC + mZ*56
            idxf = tr("idxf")
            V.tensor_mul(idxf[:], fA[:], rank[:])
            tmp = tr("tmp")
            V.tensor_mul(tmp[:], fB[:], idxB[:])
            V.tensor_add(idxf[:], idxf[:], tmp[:])
            V.tensor_mul(tmp[:], fC[:], idxC[:])
            V.tensor_add(idxf[:], idxf[:], tmp[:])
            V.tensor_scalar(tmp[:], mZ[:], 56.0, 0.0, op0=op.mult, op1=op.add)
            V.tensor_add(idxf[:], idxf[:], tmp[:])

            # Xq = sgf*(0.5*fB) + va*fA + vC*fC   (sgf = sg*fo, va = sg*mask2)
            fbh = tr("fbh")
            V.tensor_scalar(fbh[:], fB[:], 0.5, 0.0, op0=op.mult, op1=op.add)
            u1 = t8("u1")
            V.tensor_tensor(u1[:], sgf[:], bc8(fbh), op=op.mult)
            u2 = t8("u2")
            V.tensor_tensor(u2[:], va[:], bc8(fA), op=op.mult)
            xqt = out_pool.tile([P, R, D], f32, tag="xqt", name="xqt")
            V.tensor_add(xqt[:], u1[:], u2[:])
            # vC from triple/last masks (TRIPLES membership per dim)
            vc = t8("vc")
            V.tensor_add(vc[:, :, 0], mb[:, :, 0], mb[:, :, 1])
            V.tensor_add(vc[:, :, 0], vc[:, :, 0], mb[:, :, 2])
            V.tensor_add(vc[:, :, 1], mb[:, :, 0], mb[:, :, 1])
            V.tensor_add(vc[:, :, 1], vc[:, :, 1], mb[:, :, 4])
            V.tensor_add(vc[:, :, 2], mb[:, :, 0], mb[:, :, 3])
            V.tensor_add(vc[:, :, 2], vc[:, :, 2], mb[:, :, 4])
            V.tensor_add(vc[:, :, 3], mb[:, :, 2], mb[:, :, 3])
            V.tensor_add(vc[:, :, 3], vc[:, :, 3], mb[:, :, 4])
            V.tensor_add(vc[:, :, 4], mb[:, :, 1], mb[:, :, 2])
            V.tensor_add(vc[:, :, 4], vc[:, :, 4], mb[:, :, 3])
            V.tensor_copy(vc[:, :, 5:8], mb[:, :, 5:8])
            V.tensor_tensor(vc[:], vc[:], bc8(fC), op=op.mult)
            V.tensor_add(xqt[:], xqt[:], vc[:])

            idx8 = out_pool.tile([P, R], u8, tag="idx8", name="idx8")
            A.copy(idx8[:], idxf[:])

            nc.sync.dma_start(xqv[:, blk * R : (blk + 1) * R, :], xqt[:])
            nc.sync.dma_start(idxv[:, blk * R : (blk + 1) * R], idx8[:])

    nc.compile()
    return nc


def _tables_v3():
    cst = np.zeros((P, 32), np.float32)
    cst[:, 0:5] = np.arange(5, dtype=np.float32)  # tsel weights
    cst[:, 5:8] = 241.0 + 5.0 * np.arange(3, dtype=np.float32)  # lsel weights
    cst[:, 8:16] = np.array([64, 32, 16, 8, 4, 2, 1, 0], np.float32)
    cst[:, 16:24] = np.arange(8, dtype=np.float32)
    cst[:, 24:32] = np.arange(8, dtype=np.float32) ** 2
    return cst


_nc_cache = {}
LAST_RESULT = None


def _get_program(n_rows):
    if n_rows not in _nc_cache:
        _nc_cache[n_rows] = _build_program_v3(n_rows)
    return _nc_cache[n_rows]


def kernel(X, grid, grid_norm):
    from concourse.bass_utils import run_bass_kernel_spmd

    X = np.ascontiguousarray(np.asarray(X, dtype=np.float32))
    grid = np.asarray(grid, dtype=np.float32)
    assert X.shape == (N_TOTAL, D)

    cst = _tables_v3()
    nc = _get_program(N_PER_CORE)

    in_maps = []
    for c in range(N_CORES):
        shard = X[c * N_PER_CORE : (c + 1) * N_PER_CORE]
        in_maps.append({"x": np.ascontiguousarray(shard), "cst": cst})

    res = run_bass_kernel_spmd(nc, in_maps, list(range(N_CORES)))
    global LAST_RESULT
    LAST_RESULT = res

    xq_full = np.empty((N_TOTAL, D), np.float32)
    idx_full = np.empty((N_TOTAL,), np.uint8)
    for c in range(N_CORES):
        r = res.results[c]
        xq_full[c * N_PER_CORE : (c + 1) * N_PER_CORE] = r["xq"]
        idx_full[c * N_PER_CORE : (c + 1) * N_PER_CORE] = r["idx_out"]
    return xq_full, idx_full


# revision 34
# speedup vs baseline: 1.0881x; 1.0881x over previous
"""VQ codebook (E8P-style, 256 codes, dim 8) nearest-neighbor kernel for TRN2.

Contract: kernel(**inputs) takes FULL inputs (X [1048576, 8] f32, grid
[256, 8] f32, grid_norm [256] f32) and returns (Xq [1048576, 8] f32,
idx [1048576] uint8), matching the reference
    scores = 2 X @ grid.T - grid_norm ; idx = argmax(scores) ; Xq = grid[idx].

Sharding: data-parallel over the leading N dimension across 8 NeuronCores
(hardcoded). Each core runs the same SPMD program on its 131072-row shard;
results are concatenated on the host.

Algorithm — fully-arithmetic algebraic E8P decode (no 256-wide matmul, no
scans, no gathers). The codebook from reference._build_grid() is exactly
  ints  (113): entries {-1,0,1}, 0 or 2 nonzeros, even sum      idx 0..112
  halfs (128): all +-0.5 with an even number of minus signs     idx 113..240
  extras (15): four-ones masks {5 triples} x {last in 5,6,7}    idx 241..255
so per row the argmax reduces to 4 family candidates computed with cheap
row-wise vector ops:
  A0: zero code, score 0, idx 56
  A2: +-1 at the top-2 |x| positions, score 2(t1+t2)-2
  B : 0.5*sign(x) with a parity flip at argmin|x|,
      score sum|x| - 2 - (odd ? 2 min|x| : 0)
  C : score 2(maxT+maxL) - 4 over the 5 triples / 3 last dims
Both outputs are then reconstructed arithmetically:
  idx: closed-form table ranks (cartesian-order rank formulas validated
       against the table) selected by family;
  Xq : the winning code vector rebuilt from signs/masks.

Layout: partition p owns rows [p*1024, (p+1)*1024); blocks of R=128 rows
per partition are processed with [128, R, 8]-shaped fp32 ops so every DVE
instruction covers 16384 rows (instruction overhead amortized).
"""

import numpy as np

N_CORES = 8
N_TOTAL = 1048576
N_PER_CORE = N_TOTAL // N_CORES  # 131072
P = 128
D = 8


def _build_program_v3(n_rows, R=128):
    import concourse.bass as bass
    import concourse.bacc as bacc
    import concourse.mybir as mybir
    import concourse.tile as tile
    from concourse.alu_op_type import AluOpType as op
    from contextlib import ExitStack

    f32 = mybir.dt.float32
    i32 = mybir.dt.int32
    u8 = mybir.dt.uint8
    AF = mybir.ActivationFunctionType
    X_ = mybir.AxisListType.X

    rpp = n_rows // P
    assert n_rows % P == 0 and rpp % R == 0
    n_blocks = rpp // R

    nc = bacc.Bacc(trn_type="TRN2", target_bir_lowering=False, debug=False)

    x = nc.dram_tensor("x", [n_rows, D], f32, kind="ExternalInput").ap()
    # consts [128, 32]: 0:8 = WC (idxC weights), 8:16 = W7 (idxB weights),
    # 16:24 = iota, 24:32 = iota^2
    cst = nc.dram_tensor("cst", [P, 32], f32, kind="ExternalInput").ap()

    xq = nc.dram_tensor("xq", [n_rows, D], f32, kind="ExternalOutput").ap()
    idx_out = nc.dram_tensor("idx_out", [n_rows], u8, kind="ExternalOutput").ap()

    xv = x.rearrange("(p r) d -> p r d", p=P)
    xqv = xq.rearrange("(p r) d -> p r d", p=P)
    idxv = idx_out.rearrange("(p r) -> p r", p=P)

    cst_sb = nc.alloc_sbuf_tensor("cst_sb", [P, 32], f32).ap()
    wcc = cst_sb[:, 0:8]
    w7c = cst_sb[:, 8:16]
    ioc = cst_sb[:, 16:24]
    io2c = cst_sb[:, 24:32]

    def bc8(t):  # [P, R] -> [P, R, 8]
        return t.unsqueeze(2).to_broadcast([P, R, 8])

    def cc8(c):  # [P, 8] const -> [P, R, 8]
        return c.unsqueeze(1).to_broadcast([P, R, 8])

    with tile.TileContext(nc) as tc, ExitStack() as ctx:
        nc.sync.dma_start(cst_sb[:], cst[:])

        xb_pool = ctx.enter_context(tc.tile_pool(name="xb", bufs=2))
        v8_pool = ctx.enter_context(tc.tile_pool(name="v8", bufs=1))
        sc_pool = ctx.enter_context(tc.tile_pool(name="sc", bufs=1))
        out_pool = ctx.enter_context(tc.tile_pool(name="outp", bufs=2))

        V = nc.vector
        A = nc.scalar
        G = nc.gpsimd

        for blk in range(n_blocks):
            xb = xb_pool.tile([P, R, D], f32)
            nc.sync.dma_start(xb[:], xv[:, blk * R : (blk + 1) * R, :])

            def t8(tag, bufs=1):
                return v8_pool.tile(
                    [P, R, D], f32, tag=tag, name=f"t8_{tag}", bufs=bufs
                )

            def tr(tag):
                return sc_pool.tile([P, R], f32, tag=tag, name=f"tr_{tag}")

            ax = t8("ax", bufs=2)
            A.activation(ax[:], xb[:], AF.Abs)

            sumabs = tr("sumabs")
            V.tensor_reduce(sumabs[:], ax[:], axis=X_, op=op.add)
            t1 = tr("t1")
            V.tensor_reduce(t1[:], ax[:], axis=X_, op=op.max)
            mn = tr("mn")
            V.tensor_reduce(mn[:], ax[:], axis=X_, op=op.min)

            neg = t8("neg")
            V.tensor_single_scalar(neg[:], xb[:], 0.0, op=op.is_lt)
            nneg = tr("nneg")
            V.tensor_reduce(nneg[:], neg[:], axis=X_, op=op.add)
            oddi = sc_pool.tile([P, R], i32, tag="oddi", name="oddi")
            V.tensor_copy(oddi[:], nneg[:])
            V.tensor_single_scalar(oddi[:], oddi[:], 1, op=op.bitwise_and)
            odd = tr("odd")
            V.tensor_copy(odd[:], oddi[:])

            # top-2 of |x|
            ltm = t8("ltm")
            V.tensor_tensor(ltm[:], ax[:], bc8(t1), op=op.is_lt)
            axl = t8("axl", bufs=2)
            G.tensor_tensor(axl[:], ax[:], ltm[:], op=op.mult)
            t2 = tr("t2")
            V.tensor_reduce(t2[:], axl[:], axis=X_, op=op.max)
            mask2 = t8("mask2")
            V.tensor_tensor(mask2[:], ax[:], bc8(t2), op=op.is_ge)

            sA2 = tr("sA2")
            V.tensor_add(sA2[:], t1[:], t2[:])
            V.tensor_scalar(sA2[:], sA2[:], 2.0, -2.0, op0=op.mult, op1=op.add)

            sB = tr("sB")
            V.tensor_mul(sB[:], odd[:], mn[:])
            V.tensor_scalar(sB[:], sB[:], -2.0, -2.0, op0=op.mult, op1=op.add)
            V.tensor_add(sB[:], sB[:], sumabs[:])

            # C family: triple sums in tb[0:5], scratch in tb[5:8]
            tb = t8("tb")
            G.tensor_add(tb[:, :, 5], xb[:, :, 0], xb[:, :, 1])
            G.tensor_add(tb[:, :, 6], xb[:, :, 3], xb[:, :, 4])
            G.tensor_add(tb[:, :, 7], xb[:, :, 1], xb[:, :, 2])
            G.tensor_add(tb[:, :, 0], tb[:, :, 5], xb[:, :, 2])
            G.tensor_add(tb[:, :, 1], tb[:, :, 5], xb[:, :, 4])
            G.tensor_add(tb[:, :, 2], tb[:, :, 6], xb[:, :, 0])
            G.tensor_add(tb[:, :, 3], tb[:, :, 6], xb[:, :, 2])
            G.tensor_add(tb[:, :, 4], tb[:, :, 7], xb[:, :, 3])
            maxT = tr("maxT")
            V.tensor_reduce(maxT[:], tb[:, :, 0:5], axis=X_, op=op.max)
            maxL = tr("maxL")
            V.tensor_reduce(maxL[:], xb[:, :, 5:8], axis=X_, op=op.max)
            mb = t8("mb")
            V.tensor_tensor(
                mb[:, :, 0:5], tb[:, :, 0:5], bc8(maxT)[:, :, 0:5], op=op.is_ge
            )
            V.tensor_tensor(
                mb[:, :, 5:8], xb[:, :, 5:8], bc8(maxL)[:, :, 5:8], op=op.is_ge
            )
            wb = t8("wb")
            V.tensor_tensor(wb[:], mb[:], cc8(wcc), op=op.mult)
            idxC = tr("idxC")
            V.tensor_reduce(idxC[:], wb[:], axis=X_, op=op.add)
            sC = tr("sC")
            V.tensor_add(sC[:], maxT[:], maxL[:])
            V.tensor_scalar(sC[:], sC[:], 2.0, -4.0, op0=op.mult, op1=op.add)

            # winner score
            w = tr("w")
            V.tensor_max(w[:], sA2[:], sB[:])
            V.tensor_max(w[:], w[:], sC[:])
            V.tensor_single_scalar(w[:], w[:], 0.0, op=op.max)

            # sigma & parity flip (affines on ACT, plain mults on Pool)
            sg = t8("sg", bufs=2)
            A.activation(sg[:], neg[:], AF.Copy, bias=1.0, scale=-2.0)
            flip = t8("flip")
            V.tensor_tensor(flip[:], ax[:], bc8(mn), op=op.is_equal)
            fo = t8("fo", bufs=2)
            V.tensor_tensor(fo[:], flip[:], bc8(odd), op=op.mult)
            A.activation(fo[:], fo[:], AF.Copy, bias=1.0, scale=-2.0)
            sgf = t8("sgf", bufs=2)
            G.tensor_tensor(sgf[:], sg[:], fo[:], op=op.mult)
            va = t8("va", bufs=2)
            G.tensor_tensor(va[:], sg[:], mask2[:], op=op.mult)

            # family masks, tie priority A0 > A2 > B > C
            mA = tr("mA")
            V.tensor_tensor(mA[:], sA2[:], w[:], op=op.is_ge)
            mB = tr("mB")
            V.tensor_tensor(mB[:], sB[:], w[:], op=op.is_ge)
            mZ = tr("mZ")
            V.tensor_single_scalar(mZ[:], w[:], 0.0, op=op.is_le)
            nA = tr("nA")
            V.tensor_scalar(nA[:], mA[:], -1.0, 1.0, op0=op.mult, op1=op.add)
            nB = tr("nB")
            V.tensor_scalar(nB[:], mB[:], -1.0, 1.0, op0=op.mult, op1=op.add)
            nZ = tr("nZ")
            V.tensor_scalar(nZ[:], mZ[:], -1.0, 1.0, op0=op.mult, op1=op.add)
            fA = tr("fA")
            V.tensor_mul(fA[:], mA[:], nZ[:])
            fB = tr("fB")
            V.tensor_mul(fB[:], mB[:], nA[:])
            V.tensor_mul(fB[:], fB[:], nZ[:])
            fC = tr("fC")
            V.tensor_mul(fC[:], nB[:], nA[:])
            V.tensor_mul(fC[:], fC[:], nZ[:])

            # idxB = 113 + dot((sgf+1)/2 [0:7], 2^(6-d))
            #      = 0.5*dot(sgf, W7) + 176.5
            kb8 = t8("kb8")
            V.tensor_tensor(kb8[:], sgf[:], cc8(w7c), op=op.mult)
            idxB = tr("idxB")
            V.tensor_reduce(idxB[:], kb8[:], axis=X_, op=op.add)
            V.tensor_scalar(idxB[:], idxB[:], 0.5, 176.5, op0=op.mult, op1=op.add)

            # idxA: positions/signs of the top-2, closed-form rank
            dots = t8("dots")
            V.tensor_tensor(dots[:], mask2[:], cc8(ioc), op=op.mult)
            S = tr("S")
            V.tensor_reduce(S[:], dots[:], axis=X_, op=op.add)
            V.tensor_tensor(dots[:], mask2[:], cc8(io2c), op=op.mult)
            Q = tr("Q")
            V.tensor_reduce(Q[:], dots[:], axis=X_, op=op.add)
            sumva = tr("sumva")
            V.tensor_reduce(sumva[:], va[:], axis=X_, op=op.add)
            V.tensor_tensor(dots[:], va[:], cc8(ioc), op=op.mult)
            dif = tr("dif")
            V.tensor_reduce(dif[:], dots[:], axis=X_, op=op.add)

            disc = tr("disc")
            V.tensor_mul(disc[:], S[:], S[:])
            V.tensor_scalar(disc[:], disc[:], -1.0, 0.0, op0=op.mult, op1=op.add)
            qq = tr("qq")
            V.tensor_scalar(qq[:], Q[:], 2.0, 0.0, op0=op.mult, op1=op.add)
            V.tensor_add(disc[:], disc[:], qq[:])
            V.tensor_single_scalar(disc[:], disc[:], 0.0, op=op.max)
            Dd = tr("Dd")
            A.activation(Dd[:], disc[:], AF.Sqrt)
            aa = tr("aa")
            V.tensor_sub(aa[:], S[:], Dd[:])
            V.tensor_scalar(aa[:], aa[:], 0.5, 0.0, op0=op.mult, op1=op.add)
            bb = tr("bb")
            V.tensor_add(bb[:], S[:], Dd[:])
            V.tensor_scalar(bb[:], bb[:], 0.5, 0.0, op0=op.mult, op1=op.add)

            sv2 = tr("sv2")
            V.tensor_mul(sv2[:], sumva[:], sumva[:])
            msame = tr("msame")
            V.tensor_single_scalar(msame[:], sv2[:], 4.0, op=op.is_ge)
            sopp = tr("sopp")
            V.tensor_single_scalar(sopp[:], dif[:], 0.0, op=op.is_ge)
            V.tensor_scalar(sopp[:], sopp[:], -2.0, 1.0, op0=op.mult, op1=op.add)
            hsv = tr("hsv")
            V.tensor_scalar(hsv[:], sumva[:], 0.5, 0.0, op0=op.mult, op1=op.add)
            V.tensor_sub(hsv[:], hsv[:], sopp[:])
            V.tensor_mul(hsv[:], hsv[:], msame[:])
            sa = tr("sa")
            V.tensor_add(sa[:], sopp[:], hsv[:])
            sb_ = tr("sb_")
            V.tensor_sub(sb_[:], sumva[:], sa[:])
            spa = tr("spa")
            V.tensor_single_scalar(spa[:], sa[:], 0.0, op=op.is_gt)
            spb = tr("spb")
            V.tensor_single_scalar(spb[:], sb_[:], 0.0, op=op.is_gt)

            # rank = a(15-a) + spa*(2(7-a) + 2(7-a)(6-a) + 1)
            #        + (b-a-1) + spb*(1 + 2(7-b))
            f15 = tr("f15")
            V.tensor_scalar(f15[:], aa[:], -1.0, 15.0, op0=op.mult, op1=op.add)
            rank = tr("rank")
            V.tensor_mul(rank[:], aa[:], f15[:])
            s7a = tr("s7a")
            V.tensor_scalar(s7a[:], aa[:], -1.0, 7.0, op0=op.mult, op1=op.add)
            s6a = tr("s6a")
            V.tensor_scalar(s6a[:], aa[:], -1.0, 6.0, op0=op.mult, op1=op.add)
            pa = tr("pa")
            V.tensor_mul(pa[:], s7a[:], s6a[:])
            V.tensor_add(pa[:], pa[:], s7a[:])
            V.tensor_scalar(pa[:], pa[:], 2.0, 1.0, op0=op.mult, op1=op.add)
            V.tensor_mul(pa[:], pa[:], spa[:])
            V.tensor_add(rank[:], rank[:], pa[:])
            p3 = tr("p3")
            V.tensor_sub(p3[:], bb[:], aa[:])
            V.tensor_scalar(p3[:], p3[:], 1.0, -1.0, op0=op.mult, op1=op.add)
            V.tensor_add(rank[:], rank[:], p3[:])
            pb = tr("pb")
            V.tensor_scalar(pb[:], bb[:], -2.0, 15.0, op0=op.mult, op1=op.add)
            V.tensor_mul(pb[:], pb[:], spb[:])
            V.tensor_add(rank[:], rank[:], pb[:])

            # final idx = fA*rank + fB*idxB + fC*idxC + mZ*56
            idxf = tr("idxf")
            V.tensor_mul(idxf[:], fA[:], rank[:])
            tmp = tr("tmp")
            V.tensor_mul(tmp[:], fB[:], idxB[:])
            V.tensor_add(idxf[:], idxf[:], tmp[:])
            V.tensor_mul(tmp[:], fC[:], idxC[:])
            V.tensor_add(idxf[:], idxf[:], tmp[:])
            V.tensor_scalar(tmp[:], mZ[:], 56.0, 0.0, op0=op.mult, op1=op.add)
            V.tensor_add(idxf[:], idxf[:], tmp[:])

            # Xq = sgf*(0.5*fB) + va*fA + vC*fC   (sgf = sg*fo, va = sg*mask2)
            fbh = tr("fbh")
            V.tensor_scalar(fbh[:], fB[:], 0.5, 0.0, op0=op.mult, op1=op.add)
            u1 = t8("u1")
            V.tensor_tensor(u1[:], sgf[:], bc8(fbh), op=op.mult)
            u2 = t8("u2")
            V.tensor_tensor(u2[:], va[:], bc8(fA), op=op.mult)
            xqt = out_pool.tile([P, R, D], f32, tag="xqt", name="xqt")
            V.tensor_add(xqt[:], u1[:], u2[:])
            # vC from triple/last masks (TRIPLES membership per dim)
            vc = t8("vc")
            G.tensor_add(vc[:, :, 0], mb[:, :, 0], mb[:, :, 1])
            G.tensor_add(vc[:, :, 0], vc[:, :, 0], mb[:, :, 2])
            G.tensor_add(vc[:, :, 1], mb[:, :, 0], mb[:, :, 1])
            G.tensor_add(vc[:, :, 1], vc[:, :, 1], mb[:, :, 4])
            G.tensor_add(vc[:, :, 2], mb[:, :, 0], mb[:, :, 3])
            G.tensor_add(vc[:, :, 2], vc[:, :, 2], mb[:, :, 4])
            G.tensor_add(vc[:, :, 3], mb[:, :, 2], mb[:, :, 3])
            G.tensor_add(vc[:, :, 3], vc[:, :, 3], mb[:, :, 4])
            G.tensor_add(vc[:, :, 4], mb[:, :, 1], mb[:, :, 2])
            G.tensor_add(vc[:, :, 4], vc[:, :, 4], mb[:, :, 3])
            G.tensor_copy(vc[:, :, 5:8], mb[:, :, 5:8])
            V.tensor_tensor(vc[:], vc[:], bc8(fC), op=op.mult)
            V.tensor_add(xqt[:], xqt[:], vc[:])

            idx8 = out_pool.tile([P, R], u8, tag="idx8", name="idx8")
            A.copy(idx8[:], idxf[:])

            nc.sync.dma_start(xqv[:, blk * R : (blk + 1) * R, :], xqt[:])
            nc.sync.dma_start(idxv[:, blk * R : (blk + 1) * R], idx8[:])

    nc.compile()
    return nc


def _tables_v3():
    cst = np.zeros((P, 32), np.float32)
    cst[:, 0:5] = np.arange(5, dtype=np.float32)  # tsel weights
    cst[:, 5:8] = 241.0 + 5.0 * np.arange(3, dtype=np.float32)  # lsel weights
    cst[:, 8:16] = np.array([64, 32, 16, 8, 4, 2, 1, 0], np.float32)
    cst[:, 16:24] = np.arange(8, dtype=np.float32)
    cst[:, 24:32] = np.arange(8, dtype=np.float32) ** 2
    return cst


_nc_cache = {}
LAST_RESULT = None


def _get_program(n_rows):
    if n_rows not in _nc_cache:
        _nc_cache[n_rows] = _build_program_v3(n_rows)
    return _nc_cache[n_rows]


def kernel(X, grid, grid_norm):
    from concourse.bass_utils import run_bass_kernel_spmd

    X = np.ascontiguousarray(np.asarray(X, dtype=np.float32))
    grid = np.asarray(grid, dtype=np.float32)
    assert X.shape == (N_TOTAL, D)

    cst = _tables_v3()
    nc = _get_program(N_PER_CORE)

    in_maps = []
    for c in range(N_CORES):
        shard = X[c * N_PER_CORE : (c + 1) * N_PER_CORE]
        in_maps.append({"x": np.ascontiguousarray(shard), "cst": cst})

    res = run_bass_kernel_spmd(nc, in_maps, list(range(N_CORES)))
    global LAST_RESULT
    LAST_RESULT = res

    xq_full = np.empty((N_TOTAL, D), np.float32)
    idx_full = np.empty((N_TOTAL,), np.uint8)
    for c in range(N_CORES):
        r = res.results[c]
        xq_full[c * N_PER_CORE : (c + 1) * N_PER_CORE] = r["xq"]
        idx_full[c * N_PER_CORE : (c + 1) * N_PER_CORE] = r["idx_out"]
    return xq_full, idx_full
